# revision 24
# baseline (speedup 1.0000x reference)
"""Trainium2 Bass kernel for nn_BoundaryKDV7 (boundary KL-divergence loss), v2.

Contract: kernel(**inputs) takes the FULL inputs
    preds_S [8, 14, 512, 512] f32
    preds_T [8, 14, 512, 512] f32
    gt_labels [8, 1, 512, 512] i32
and returns the scalar f32 loss. Batch dim sharded across 8 NeuronCores
(pure data parallel); each core emits per-class per-column partial sums
which the host reduces to the scalar.

Math (matches the reference up to fp reassociation):
  boundary_k = (gt == k) & (any 4-neighbor label != k, -1-padded border)
  kl_pix = W/ZT + lnZS - lnZT, with
    ZT = sum_c exp(t_c), ZS = sum_c exp(s_c), W = sum_c exp(t_c)(t_c - s_c)
  n_k   = sum_p boundary_k
  kls_k = sum_p boundary_k * kl_pix
  valid_k = n_k > [gt[0,0] == k]
  loss = sum_{b,k} valid * kls / (14 * max(n, 1))

v2 changes vs v1:
  - inputs are cast f32 -> f16 during the DMA itself (SWDGE cast DMA), so
    every DVE elementwise op downstream runs in 16-bit 2x/4x perf modes.
  - processing unit is a half-superchunk [112, 4096] (32768 px): partition
    p = c*8 + j holds pixels 32768g + 4096j + f for global half g in 0..7.
  - per 65536-px superchunk the channel sums (ZT, ZS, W) accumulate into
    one pixel-major [128, 512] PSUM plane each via constant selector
    matmuls: psum row r = 64h + 16j + u, col v, where f = 512u + v
    (<=> psum[r, v] is pixel 65536s + 512r + v, image row 128s + r).
  - boundary via 3 max ops instead of sum+threshold (one DVE op fewer).
  - phase C: per superchunk, 13 masks (gtv == k) on DVE 4x mode, 13
    products mask*kl on DVE 2x mode, all column-reduced over partitions on
    the TensorEngine via indicator-column matmuls into one [26, 512] PSUM
    tile (row k-1 = counts, row 13+k-1 = kl sums); host sums the columns.
"""

import numpy as np
from contextlib import ExitStack

B, C, H, W = 8, 14, 512, 512
P = H * W              # 262144 pixels per sample
K = C - 1              # 13 foreground classes
NSC = 4                # superchunks (65536 px each)
NG = 8                 # half-superchunks (32768 px each)
HF = 4096              # free dim of a half-superchunk tile
NV = 2 * K             # 26 reduced value rows
N_CORES = 8

_CACHE = {}


def _build_sel() -> np.ndarray:
    """Phase-B selector weights [112, 16*128] f16.

    Block blk = 8*h + u is the lhsT for (half h of the superchunk, 512-col
    chunk u of the half-tile): sel[(c*8+j), blk, r] = 1 iff
    r == 64*h + 8*j + u, mapping tile column 512*u + v of partition
    (c, j) (pixel 32768*(2s+h) + 4096*j + 512*u + v) to psum row r, col v.
    """
    sel = np.zeros((112, 16, 128), np.float16)
    for h in range(2):
        for u in range(8):
            blk = 8 * h + u
            for j in range(8):
                row = 64 * h + 8 * j + u
                sel[j::8, blk, row] = 1.0
    return np.ascontiguousarray(sel.reshape(112, 16 * 128))


def _build_selc() -> np.ndarray:
    """Phase-C indicator columns [128, 26*26] f16: block v has column v
    all-ones, so matmul(lhsT=block_v, rhs=plane) adds the per-column
    partition sums of `plane` into row v of the [26, 512] PSUM tile."""
    selc = np.zeros((128, NV, NV), np.float16)
    for v in range(NV):
        selc[:, v, v] = 1.0
    return np.ascontiguousarray(selc.reshape(128, NV * NV))


def _patched_act_tables(orig_fn):
    """Force Exp and Ln to resolve to the one table set containing both
    (natural_log_exp_and_others) so the kernel never switches sets."""
    def wrapper(arch):
        import concourse.mybir as mybir
        tabs = orig_fn(arch)
        both = "natural_log_exp_and_others"
        if both in tabs:
            for name, funcs in tabs.items():
                if name != both:
                    funcs.discard(mybir.ActivationFunctionType.Exp)
                    funcs.discard(mybir.ActivationFunctionType.Ln)
        return tabs
    return wrapper


def _emit(nc, tc, S, T, GT, SEL, SELC, OUT):
    import concourse.bass as bass
    from concourse import mybir

    f32 = mybir.dt.float32
    f16 = mybir.dt.float16
    Alu = mybir.AluOpType
    Act = mybir.ActivationFunctionType

    with ExitStack() as ctx:
        consts = ctx.enter_context(tc.tile_pool(name="consts", bufs=1))
        planes = ctx.enter_context(tc.tile_pool(name="planes", bufs=1))
        scratch = ctx.enter_context(tc.tile_pool(name="scratch", bufs=5))
        inpool = ctx.enter_context(tc.tile_pool(name="inpool", bufs=3))
        midpool = ctx.enter_context(tc.tile_pool(name="midpool", bufs=2))
        finpool = ctx.enter_context(tc.tile_pool(name="finpool", bufs=2))
        cpool = ctx.enter_context(tc.tile_pool(name="cpool", bufs=1))
        mkpool = ctx.enter_context(tc.tile_pool(name="mkpool", bufs=2))
        psum = ctx.enter_context(
            tc.tile_pool(name="psum", bufs=2, space=bass.MemorySpace.PSUM))
        psumc = ctx.enter_context(
            tc.tile_pool(name="psumc", bufs=1, space=bass.MemorySpace.PSUM))

        # ---- constants ----
        sel_sb = consts.tile([112, 16 * 128], f16)
        nc.sync.dma_start(sel_sb[:], SEL[:])
        selc_sb = consts.tile([128, NV * NV], f16)
        nc.sync.dma_start(selc_sb[:], SELC[:])

        # ---- gt halo load: G[r', s, t, f] = gt_pad[128*s + r' + t, f] ----
        G = consts.tile([128, 4, 3, W + 4], f16)
        for t in range(3):
            nc.sync.dma_start(
                G[:, :, t, :],
                GT[t:t + 512, :].rearrange("(s r) f -> r s f", s=4))

        # ---- phase A: boundary mask ----
        Cv = G[:, :, 1, 2:514]   # center labels  [128, 4, 512]
        Uv = G[:, :, 0, 2:514]
        Dv = G[:, :, 2, 2:514]
        Lv = G[:, :, 1, 1:513]
        Rv = G[:, :, 1, 3:515]

        e1 = scratch.tile([128, 4, 512], f16, tag="pa")
        e2 = scratch.tile([128, 4, 512], f16, tag="pa")
        nc.vector.tensor_tensor(e1[:], Cv, Uv, Alu.not_equal)
        nc.vector.tensor_tensor(e2[:], Cv, Dv, Alu.not_equal)
        x1 = scratch.tile([128, 4, 512], f16, tag="pa")
        nc.vector.tensor_tensor(x1[:], e1[:], e2[:], Alu.max)
        e3 = scratch.tile([128, 4, 512], f16, tag="pa")
        e4 = scratch.tile([128, 4, 512], f16, tag="pa")
        nc.vector.tensor_tensor(e3[:], Cv, Lv, Alu.not_equal)
        nc.vector.tensor_tensor(e4[:], Cv, Rv, Alu.not_equal)
        x2 = scratch.tile([128, 4, 512], f16, tag="pa")
        nc.vector.tensor_tensor(x2[:], e3[:], e4[:], Alu.max)
        dif = scratch.tile([128, 4, 512], f16, tag="pa")
        nc.vector.tensor_tensor(dif[:], x1[:], x2[:], Alu.max)
        # gtv = label * [any neighbor differs]; label-0 pixels vanish in
        # the product, so no separate (label >= 1) mask is needed
        gtv = planes.tile([128, 4, 512], f16)     # label if boundary else 0
        nc.vector.tensor_mul(gtv[:], Cv, dif[:])

        # ---- phase C reduction target: one accumulation group of
        #      26 values x 4 superchunks matmuls into [26, 512] ----
        acc = psumc.tile([NV, 512], f32)
        n_cmm = NV * NSC
        cmm = [0]  # matmul counter for start/stop flags

        def c_reduce(plane, v):
            st = cmm[0] == 0
            sp = cmm[0] == n_cmm - 1
            nc.tensor.matmul(acc[:], selc_sb[:, v * NV:(v + 1) * NV],
                             plane, start=st, stop=sp)
            cmm[0] += 1

        # ---- phase B: softmax KL (+ phase C interleaved) ----
        # global half g in 0..7: pixels [32768g, 32768(g+1))
        Sr = S.rearrange("c (g j f) -> g c j f", j=8, f=HF)
        Tr = T.rearrange("c (g j f) -> g c j f", j=8, f=HF)

        def emit_half(g, psZT, psZS, psW, nsplit=1):
            """DMA + exp/sub/mul + the ZT/ZS/W matmul groups for half g.
            nsplit > 1 splits the DMA and elementwise work into that many
            column sub-slices (used for the final half-chunks so the tail
            dependency chain after the last HBM byte is short)."""
            hh = g % 2
            St = inpool.tile([112, HF], f16, tag="St")
            Tt = inpool.tile([112, HF], f16, tag="Tt")
            eT = midpool.tile([112, HF], f16, tag="eT")
            eS = midpool.tile([112, HF], f16, tag="eS")
            d = midpool.tile([112, HF], f16, tag="d")
            w = HF // nsplit
            for q in range(nsplit):
                cs = slice(q * w, (q + 1) * w)
                nc.gpsimd.dma_start(St[:, cs], Sr[g][:, :, cs])  # f32->f16
                nc.gpsimd.dma_start(Tt[:, cs], Tr[g][:, :, cs])
            for q in range(nsplit):
                cs = slice(q * w, (q + 1) * w)
                nc.vector.tensor_sub(d[:, cs], Tt[:, cs], St[:, cs])
                nc.scalar.activation(eT[:, cs], Tt[:, cs], Act.Exp)
                nc.scalar.activation(eS[:, cs], St[:, cs], Act.Exp)
                # m = eT * d computed in place over d (stream-safe)
                nc.vector.tensor_mul(d[:, cs], eT[:, cs], d[:, cs])

            def mm_group(plane, ps):
                for u in range(8):
                    blk = 8 * hh + u
                    selap = sel_sb[:, blk * 128:(blk + 1) * 128]
                    st = (hh == 0 and u == 0)
                    sp = (hh == 1 and u == 7)
                    cs = slice(u * 512, (u + 1) * 512)
                    nc.tensor.matmul(ps[:], selap, plane[:, cs],
                                     start=st, stop=sp)

            mm_group(eT, psZT)
            mm_group(eS, psZS)
            mm_group(d, psW)

        def make_masks_counts(s, tag="mka", bufs=None):
            """Masks for superchunk s (need only gtv) + count reduces.
            All 13 class masks live in one [128, 13, 512] tile so the
            later mask*kl product is a single wide DVE op."""
            mka = mkpool.tile([128, K, 512], f16, tag=tag, bufs=bufs)
            for k in range(1, C):
                nc.vector.tensor_single_scalar(mka[:, k - 1, :],
                                               gtv[:, s, :],
                                               float(k), Alu.is_equal)
                c_reduce(mka[:, k - 1, :], k - 1)
            return mka

        # the last superchunk's masks + count reduces run in the startup
        # shadow (they need only gtv) so the kernel tail never waits on
        # mask generation or count matmuls
        mka_last = make_masks_counts(NSC - 1, tag="mka3", bufs=1)

        def make_finals_parts(s, mka, psZT, psZS, psW):
            """Finals of superchunk s as closures, interleaved into the
            next superchunk's window (their inputs are ready by then)."""
            st = {}

            def part_act():
                # order LnZT -> r -> LnZS: h (needs r) and g (needs lnZS)
                # unblock at nearly the same time
                lnZT = finpool.tile([128, 512], f16, tag="lnZT")
                lnZS = finpool.tile([128, 512], f16, tag="lnZS")
                r = finpool.tile([128, 512], f16, tag="r")
                nc.scalar.activation(lnZT[:], psZT[:], Act.Ln)
                nc.scalar.activation(r[:], lnZT[:], Act.Exp, scale=-1.0)
                nc.scalar.activation(lnZS[:], psZS[:], Act.Ln)
                st["lnZT"], st["lnZS"], st["r"] = lnZT, lnZS, r

            def part_kl():
                g_ = finpool.tile([128, 512], f16, tag="g")
                h_ = finpool.tile([128, 512], f16, tag="h")
                kl = finpool.tile([128, 512], f16, tag="kl")
                nc.vector.tensor_sub(g_[:], st["lnZS"][:], st["lnZT"][:])
                nc.vector.tensor_mul(h_[:], psW[:], st["r"][:])
                nc.vector.tensor_add(kl[:], h_[:], g_[:])
                st["kl"] = kl

            def part_pk():
                # all 13 mask*kl products in ONE wide DVE op (kl broadcast
                # along the class dim with stride 0)
                pka = cpool.tile([128, K, 512], f16, tag="pka")
                klb = st["kl"][:, None, :].broadcast_to([128, K, 512])
                nc.vector.tensor_mul(pka[:], mka[:], klb)
                st["pka"] = pka

            def part_klsum():
                for k in range(1, C):
                    c_reduce(st["pka"][:, k - 1, :], K + k - 1)

            def part_pk_interleaved():
                # tail variant: per-class mul immediately followed by its
                # reduce matmul, so DVE and PE ping-pong at ~400ns cadence
                # instead of one 3.6us mul blocking a cold-PE matmul burst
                for k in range(1, C):
                    pk = cpool.tile([128, 512], f16, tag="pks", bufs=4)
                    nc.vector.tensor_mul(pk[:], mka[:, k - 1, :],
                                         st["kl"][:])
                    c_reduce(pk[:], K + k - 1)

            return [part_act, part_kl, part_pk, part_klsum,
                    part_pk_interleaved]

        # software pipeline: superchunk s's finals/phase-C are emitted in
        # superchunk s+1's window (their inputs — the s psums — are ready
        # by then). The s+1 exps go FIRST on the ACT queue so they start
        # the moment their DMA lands; the lnZ trio of s follows them.
        pending = None
        for s in range(NSC):
            psZT = psum.tile([128, 512], f32, tag="psZT")
            psZS = psum.tile([128, 512], f32, tag="psZS")
            psW = psum.tile([128, 512], f32, tag="psW")
            if pending:
                pending[0]()        # ACT: Ln/r/Ln of s-1 (ready at head)
                pending[1]()        # DVE: g/h/kl of s-1
                pending[2]()        # DVE: fused mask*kl of s-1
                pending[3]()        # PE: 13 klsum reduces of s-1
            emit_half(2 * s, psZT, psZS, psW,
                      nsplit=(2 if s == NSC - 1 else 1))
            mka = (make_masks_counts(s) if s < NSC - 1 else mka_last)
            emit_half(2 * s + 1, psZT, psZS, psW,
                      nsplit=(2 if s == NSC - 1 else 1))
            pending = make_finals_parts(s, mka, psZT, psZS, psW)
        # tail: finals of the last superchunk with interleaved pk reduces
        pending[0]()
        pending[1]()
        pending[4]()

        acc_sb = planes.tile([NV, 512], f32)
        nc.vector.tensor_copy(acc_sb[:], acc[:])
        nc.sync.dma_start(OUT[:], acc_sb[:])


def _build_nc():
    import concourse.bacc as bacc
    import concourse.tile as tile
    import concourse.hw_specs as hw_specs
    from concourse import mybir

    if not getattr(bacc, "_act_tables_patched", False):
        bacc.get_activation_tables = _patched_act_tables(
            hw_specs.get_activation_tables)
        bacc._act_tables_patched = True

    f32 = mybir.dt.float32
    f16 = mybir.dt.float16

    nc = bacc.Bacc("TRN2", target_bir_lowering=False, debug=False)
    S = nc.declare_dram_parameter("preds_s", [C, P], f32, isOutput=False)
    T = nc.declare_dram_parameter("preds_t", [C, P], f32, isOutput=False)
    GT = nc.declare_dram_parameter("gt16", [H + 2, W + 4], f16, isOutput=False)
    SEL = nc.declare_dram_parameter("sel", [112, 16 * 128], f16,
                                    isOutput=False)
    SELC = nc.declare_dram_parameter("selc", [128, NV * NV], f16,
                                     isOutput=False)
    OUT = nc.declare_dram_parameter("partials", [NV, 512], f32, isOutput=True)
    with tile.TileContext(nc) as tc:
        _emit(nc, tc, S, T, GT, SEL, SELC, OUT)
    nc.compile()
    return nc


def _get_nc():
    if "nc" not in _CACHE:
        _CACHE["nc"] = _build_nc()
    return _CACHE["nc"]


def make_in_maps(preds_S, preds_T, gt_labels):
    """Shard the full inputs into per-core input maps (host-side layout)."""
    gt = np.asarray(gt_labels)[:, 0]                       # [nb, 512, 512]
    nb = gt.shape[0]
    gt16 = np.full((nb, H + 2, W + 4), -1.0, np.float16)
    gt16[:, 1:H + 1, 2:W + 2] = gt.astype(np.float16)
    sel = _build_sel()
    selc = _build_selc()
    pS = np.ascontiguousarray(np.asarray(preds_S, np.float32).reshape(nb, C, P))
    pT = np.ascontiguousarray(np.asarray(preds_T, np.float32).reshape(nb, C, P))
    return [
        {"preds_s": pS[b], "preds_t": pT[b], "gt16": gt16[b],
         "sel": sel, "selc": selc}
        for b in range(nb)
    ]


def postprocess(gt_labels, partials_per_core) -> np.float32:
    """Reduce per-core [26, 512] partials to the scalar loss."""
    gt = np.asarray(gt_labels)[:, 0]
    nb = gt.shape[0]
    loss = 0.0
    for b in range(nb):
        part = partials_per_core[b].astype(np.float64).sum(axis=1)  # [26]
        g0 = int(gt[b, 0, 0])
        for k in range(1, C):
            n = part[k - 1]
            kls = part[K + k - 1]
            z = 1.0 if g0 == k else 0.0
            if n > z:
                loss += kls / (C * max(n, 1.0))
    return np.float32(loss)


def _run(inputs, trace=False, trace_kwargs=None):
    from concourse.bass_utils import run_bass_kernel_spmd

    nc = _get_nc()
    in_maps = make_in_maps(inputs["preds_S"], inputs["preds_T"],
                           inputs["gt_labels"])
    res = run_bass_kernel_spmd(nc, in_maps, list(range(len(in_maps))),
                               trace=trace, **(trace_kwargs or {}))
    parts = [res.results[b]["partials"] for b in range(len(in_maps))]
    loss = postprocess(inputs["gt_labels"], parts)
    return loss, res


def kernel(preds_S, preds_T, gt_labels):
    assert preds_S.shape == (B, C, H, W), preds_S.shape
    loss, _ = _run({"preds_S": preds_S, "preds_T": preds_T,
                    "gt_labels": gt_labels})
    return loss


# revision 25
# speedup vs baseline: 1.0123x; 1.0123x over previous
"""Trainium2 Bass kernel for nn_BoundaryKDV7 (boundary KL-divergence loss), v2.

Contract: kernel(**inputs) takes the FULL inputs
    preds_S [8, 14, 512, 512] f32
    preds_T [8, 14, 512, 512] f32
    gt_labels [8, 1, 512, 512] i32
and returns the scalar f32 loss. Batch dim sharded across 8 NeuronCores
(pure data parallel); each core emits per-class per-column partial sums
which the host reduces to the scalar.

Math (matches the reference up to fp reassociation):
  boundary_k = (gt == k) & (any 4-neighbor label != k, -1-padded border)
  kl_pix = W/ZT + lnZS - lnZT, with
    ZT = sum_c exp(t_c), ZS = sum_c exp(s_c), W = sum_c exp(t_c)(t_c - s_c)
  n_k   = sum_p boundary_k
  kls_k = sum_p boundary_k * kl_pix
  valid_k = n_k > [gt[0,0] == k]
  loss = sum_{b,k} valid * kls / (14 * max(n, 1))

v2 changes vs v1:
  - inputs are cast f32 -> f16 during the DMA itself (SWDGE cast DMA), so
    every DVE elementwise op downstream runs in 16-bit 2x/4x perf modes.
  - processing unit is a half-superchunk [112, 4096] (32768 px): partition
    p = c*8 + j holds pixels 32768g + 4096j + f for global half g in 0..7.
  - per 65536-px superchunk the channel sums (ZT, ZS, W) accumulate into
    one pixel-major [128, 512] PSUM plane each via constant selector
    matmuls: psum row r = 64h + 16j + u, col v, where f = 512u + v
    (<=> psum[r, v] is pixel 65536s + 512r + v, image row 128s + r).
  - boundary via 3 max ops instead of sum+threshold (one DVE op fewer).
  - phase C: per superchunk, 13 masks (gtv == k) on DVE 4x mode, 13
    products mask*kl on DVE 2x mode, all column-reduced over partitions on
    the TensorEngine via indicator-column matmuls into one [26, 512] PSUM
    tile (row k-1 = counts, row 13+k-1 = kl sums); host sums the columns.
"""

import numpy as np
from contextlib import ExitStack

B, C, H, W = 8, 14, 512, 512
P = H * W              # 262144 pixels per sample
K = C - 1              # 13 foreground classes
NSC = 4                # superchunks (65536 px each)
NG = 8                 # half-superchunks (32768 px each)
HF = 4096              # free dim of a half-superchunk tile
NV = 2 * K             # 26 reduced value rows
N_CORES = 8

_CACHE = {}


def _build_sel() -> np.ndarray:
    """Phase-B selector weights [112, 16*128] f16.

    Block blk = 8*h + u is the lhsT for (half h of the superchunk, 512-col
    chunk u of the half-tile): sel[(c*8+j), blk, r] = 1 iff
    r == 64*h + 8*j + u, mapping tile column 512*u + v of partition
    (c, j) (pixel 32768*(2s+h) + 4096*j + 512*u + v) to psum row r, col v.
    """
    sel = np.zeros((112, 16, 128), np.float16)
    for h in range(2):
        for u in range(8):
            blk = 8 * h + u
            for j in range(8):
                row = 64 * h + 8 * j + u
                sel[j::8, blk, row] = 1.0
    return np.ascontiguousarray(sel.reshape(112, 16 * 128))


def _build_selc() -> np.ndarray:
    """Phase-C indicator columns [128, 26*26] f16: block v has column v
    all-ones, so matmul(lhsT=block_v, rhs=plane) adds the per-column
    partition sums of `plane` into row v of the [26, 512] PSUM tile."""
    selc = np.zeros((128, NV, NV), np.float16)
    for v in range(NV):
        selc[:, v, v] = 1.0
    return np.ascontiguousarray(selc.reshape(128, NV * NV))


def _patched_act_tables(orig_fn):
    """Force Exp and Ln to resolve to the one table set containing both
    (natural_log_exp_and_others) so the kernel never switches sets."""
    def wrapper(arch):
        import concourse.mybir as mybir
        tabs = orig_fn(arch)
        both = "natural_log_exp_and_others"
        if both in tabs:
            for name, funcs in tabs.items():
                if name != both:
                    funcs.discard(mybir.ActivationFunctionType.Exp)
                    funcs.discard(mybir.ActivationFunctionType.Ln)
        return tabs
    return wrapper


def _emit(nc, tc, S, T, GT, SEL, SELC, OUT):
    import concourse.bass as bass
    from concourse import mybir

    f32 = mybir.dt.float32
    f16 = mybir.dt.float16
    Alu = mybir.AluOpType
    Act = mybir.ActivationFunctionType

    with ExitStack() as ctx:
        consts = ctx.enter_context(tc.tile_pool(name="consts", bufs=1))
        planes = ctx.enter_context(tc.tile_pool(name="planes", bufs=1))
        scratch = ctx.enter_context(tc.tile_pool(name="scratch", bufs=5))
        inpool = ctx.enter_context(tc.tile_pool(name="inpool", bufs=3))
        midpool = ctx.enter_context(tc.tile_pool(name="midpool", bufs=2))
        finpool = ctx.enter_context(tc.tile_pool(name="finpool", bufs=2))
        cpool = ctx.enter_context(tc.tile_pool(name="cpool", bufs=1))
        mkpool = ctx.enter_context(tc.tile_pool(name="mkpool", bufs=2))
        psum = ctx.enter_context(
            tc.tile_pool(name="psum", bufs=2, space=bass.MemorySpace.PSUM))
        psumc = ctx.enter_context(
            tc.tile_pool(name="psumc", bufs=1, space=bass.MemorySpace.PSUM))

        # ---- constants ----
        sel_sb = consts.tile([112, 16 * 128], f16)
        nc.sync.dma_start(sel_sb[:], SEL[:])
        selc_sb = consts.tile([128, NV * NV], f16)
        nc.sync.dma_start(selc_sb[:], SELC[:])

        # ---- gt halo load: G[r', s, t, f] = gt_pad[128*s + r' + t, f] ----
        G = consts.tile([128, 4, 3, W + 4], f16)
        for t in range(3):
            nc.sync.dma_start(
                G[:, :, t, :],
                GT[t:t + 512, :].rearrange("(s r) f -> r s f", s=4))

        # ---- phase A: boundary mask ----
        Cv = G[:, :, 1, 2:514]   # center labels  [128, 4, 512]
        Uv = G[:, :, 0, 2:514]
        Dv = G[:, :, 2, 2:514]
        Lv = G[:, :, 1, 1:513]
        Rv = G[:, :, 1, 3:515]

        e1 = scratch.tile([128, 4, 512], f16, tag="pa")
        e2 = scratch.tile([128, 4, 512], f16, tag="pa")
        nc.vector.tensor_tensor(e1[:], Cv, Uv, Alu.not_equal)
        nc.vector.tensor_tensor(e2[:], Cv, Dv, Alu.not_equal)
        x1 = scratch.tile([128, 4, 512], f16, tag="pa")
        nc.vector.tensor_tensor(x1[:], e1[:], e2[:], Alu.max)
        e3 = scratch.tile([128, 4, 512], f16, tag="pa")
        e4 = scratch.tile([128, 4, 512], f16, tag="pa")
        nc.vector.tensor_tensor(e3[:], Cv, Lv, Alu.not_equal)
        nc.vector.tensor_tensor(e4[:], Cv, Rv, Alu.not_equal)
        x2 = scratch.tile([128, 4, 512], f16, tag="pa")
        nc.vector.tensor_tensor(x2[:], e3[:], e4[:], Alu.max)
        dif = scratch.tile([128, 4, 512], f16, tag="pa")
        nc.vector.tensor_tensor(dif[:], x1[:], x2[:], Alu.max)
        # gtv = label * [any neighbor differs]; label-0 pixels vanish in
        # the product, so no separate (label >= 1) mask is needed
        gtv = planes.tile([128, 4, 512], f16)     # label if boundary else 0
        nc.vector.tensor_mul(gtv[:], Cv, dif[:])

        # ---- phase C reduction target: one accumulation group of
        #      26 values x 4 superchunks matmuls into [26, 512] ----
        acc = psumc.tile([NV, 512], f32)
        n_cmm = NV * NSC
        cmm = [0]  # matmul counter for start/stop flags

        def c_reduce(plane, v):
            st = cmm[0] == 0
            sp = cmm[0] == n_cmm - 1
            nc.tensor.matmul(acc[:], selc_sb[:, v * NV:(v + 1) * NV],
                             plane, start=st, stop=sp)
            cmm[0] += 1

        # ---- phase B: softmax KL (+ phase C interleaved) ----
        # global half g in 0..7: pixels [32768g, 32768(g+1))
        Sr = S.rearrange("c (g j f) -> g c j f", j=8, f=HF)
        Tr = T.rearrange("c (g j f) -> g c j f", j=8, f=HF)

        def emit_half(g, psZT, psZS, psW, nsplit=1):
            """DMA + exp/sub/mul + the ZT/ZS/W matmul groups for half g.
            nsplit > 1 splits the DMA and elementwise work into that many
            column sub-slices (used for the final half-chunks so the tail
            dependency chain after the last HBM byte is short)."""
            hh = g % 2
            St = inpool.tile([112, HF], f16, tag="St")
            Tt = inpool.tile([112, HF], f16, tag="Tt")
            eT = midpool.tile([112, HF], f16, tag="eT")
            eS = midpool.tile([112, HF], f16, tag="eS")
            d = midpool.tile([112, HF], f16, tag="d")
            w = HF // nsplit
            for q in range(nsplit):
                cs = slice(q * w, (q + 1) * w)
                nc.gpsimd.dma_start(St[:, cs], Sr[g][:, :, cs])  # f32->f16
                nc.gpsimd.dma_start(Tt[:, cs], Tr[g][:, :, cs])
            for q in range(nsplit):
                cs = slice(q * w, (q + 1) * w)
                nc.vector.tensor_sub(d[:, cs], Tt[:, cs], St[:, cs])
                nc.scalar.activation(eT[:, cs], Tt[:, cs], Act.Exp)
                nc.scalar.activation(eS[:, cs], St[:, cs], Act.Exp)
                # m = eT * d computed in place over d (stream-safe)
                nc.vector.tensor_mul(d[:, cs], eT[:, cs], d[:, cs])

            def mm_group(plane, ps):
                for u in range(8):
                    blk = 8 * hh + u
                    selap = sel_sb[:, blk * 128:(blk + 1) * 128]
                    st = (hh == 0 and u == 0)
                    sp = (hh == 1 and u == 7)
                    cs = slice(u * 512, (u + 1) * 512)
                    nc.tensor.matmul(ps[:], selap, plane[:, cs],
                                     start=st, stop=sp)

            mm_group(eT, psZT)
            mm_group(eS, psZS)
            mm_group(d, psW)

        def make_masks_counts(s, tag="mka", bufs=None):
            """Masks for superchunk s (need only gtv) + count reduces.
            All 13 class masks live in one [128, 13, 512] tile so the
            later mask*kl product is a single wide DVE op."""
            mka = mkpool.tile([128, K, 512], f16, tag=tag, bufs=bufs)
            for k in range(1, C):
                nc.vector.tensor_single_scalar(mka[:, k - 1, :],
                                               gtv[:, s, :],
                                               float(k), Alu.is_equal)
                c_reduce(mka[:, k - 1, :], k - 1)
            return mka

        # the last superchunk's masks + count reduces run in the startup
        # shadow (they need only gtv) so the kernel tail never waits on
        # mask generation or count matmuls
        mka_last = make_masks_counts(NSC - 1, tag="mka3", bufs=1)

        def make_finals_parts(s, mka, psZT, psZS, psW):
            """Finals of superchunk s as closures, interleaved into the
            next superchunk's window (their inputs are ready by then)."""
            st = {}

            def part_act():
                # order LnZT -> r -> LnZS: h (needs r) and g (needs lnZS)
                # unblock at nearly the same time
                lnZT = finpool.tile([128, 512], f16, tag="lnZT")
                lnZS = finpool.tile([128, 512], f16, tag="lnZS")
                r = finpool.tile([128, 512], f16, tag="r")
                nc.scalar.activation(lnZT[:], psZT[:], Act.Ln)
                nc.scalar.activation(r[:], lnZT[:], Act.Exp, scale=-1.0)
                nc.scalar.activation(lnZS[:], psZS[:], Act.Ln)
                st["lnZT"], st["lnZS"], st["r"] = lnZT, lnZS, r

            def part_kl():
                g_ = finpool.tile([128, 512], f16, tag="g")
                h_ = finpool.tile([128, 512], f16, tag="h")
                kl = finpool.tile([128, 512], f16, tag="kl")
                nc.vector.tensor_sub(g_[:], st["lnZS"][:], st["lnZT"][:])
                nc.vector.tensor_mul(h_[:], psW[:], st["r"][:])
                nc.vector.tensor_add(kl[:], h_[:], g_[:])
                st["kl"] = kl

            def part_pk():
                # all 13 mask*kl products in ONE wide DVE op (kl broadcast
                # along the class dim with stride 0)
                pka = cpool.tile([128, K, 512], f16, tag="pka")
                klb = st["kl"][:, None, :].broadcast_to([128, K, 512])
                nc.vector.tensor_mul(pka[:], mka[:], klb)
                st["pka"] = pka

            def part_klsum():
                for k in range(1, C):
                    c_reduce(st["pka"][:, k - 1, :], K + k - 1)

            def part_pk_interleaved():
                # tail variant: per-class mul immediately followed by its
                # reduce matmul, so DVE and PE ping-pong at ~400ns cadence
                # instead of one 3.6us mul blocking a cold-PE matmul burst
                for k in range(1, C):
                    pk = cpool.tile([128, 512], f16, tag="pks", bufs=4)
                    nc.vector.tensor_mul(pk[:], mka[:, k - 1, :],
                                         st["kl"][:])
                    c_reduce(pk[:], K + k - 1)

            return [part_act, part_kl, part_pk, part_klsum,
                    part_pk_interleaved]

        # software pipeline: superchunk s's finals/phase-C are emitted in
        # superchunk s+1's window (their inputs — the s psums — are ready
        # by then). The s+1 exps go FIRST on the ACT queue so they start
        # the moment their DMA lands; the lnZ trio of s follows them.
        pending = None
        for s in range(NSC):
            psZT = psum.tile([128, 512], f32, tag="psZT")
            psZS = psum.tile([128, 512], f32, tag="psZS")
            psW = psum.tile([128, 512], f32, tag="psW")
            emit_half(2 * s, psZT, psZS, psW,
                      nsplit=(2 if s == NSC - 1 else 1))
            if pending:
                pending[0]()        # ACT: Ln/Ln/r of s-1
                pending[1]()        # DVE: g/h/kl of s-1
                pending[2]()        # DVE: fused mask*kl of s-1
            mka = (make_masks_counts(s) if s < NSC - 1 else mka_last)
            if pending:
                pending[3]()        # PE: 13 klsum reduces of s-1
            emit_half(2 * s + 1, psZT, psZS, psW,
                      nsplit=(2 if s == NSC - 1 else 1))
            pending = make_finals_parts(s, mka, psZT, psZS, psW)
        # tail: finals of the last superchunk with interleaved pk reduces
        pending[0]()
        pending[1]()
        pending[4]()

        acc_sb = planes.tile([NV, 512], f32)
        nc.vector.tensor_copy(acc_sb[:], acc[:])
        nc.sync.dma_start(OUT[:], acc_sb[:])


def _build_nc():
    import concourse.bacc as bacc
    import concourse.tile as tile
    import concourse.hw_specs as hw_specs
    from concourse import mybir

    if not getattr(bacc, "_act_tables_patched", False):
        bacc.get_activation_tables = _patched_act_tables(
            hw_specs.get_activation_tables)
        bacc._act_tables_patched = True

    f32 = mybir.dt.float32
    f16 = mybir.dt.float16

    nc = bacc.Bacc("TRN2", target_bir_lowering=False, debug=False)
    S = nc.declare_dram_parameter("preds_s", [C, P], f32, isOutput=False)
    T = nc.declare_dram_parameter("preds_t", [C, P], f32, isOutput=False)
    GT = nc.declare_dram_parameter("gt16", [H + 2, W + 4], f16, isOutput=False)
    SEL = nc.declare_dram_parameter("sel", [112, 16 * 128], f16,
                                    isOutput=False)
    SELC = nc.declare_dram_parameter("selc", [128, NV * NV], f16,
                                     isOutput=False)
    OUT = nc.declare_dram_parameter("partials", [NV, 512], f32, isOutput=True)
    with tile.TileContext(nc) as tc:
        _emit(nc, tc, S, T, GT, SEL, SELC, OUT)
    nc.compile()
    return nc


def _get_nc():
    if "nc" not in _CACHE:
        _CACHE["nc"] = _build_nc()
    return _CACHE["nc"]


def make_in_maps(preds_S, preds_T, gt_labels):
    """Shard the full inputs into per-core input maps (host-side layout)."""
    gt = np.asarray(gt_labels)[:, 0]                       # [nb, 512, 512]
    nb = gt.shape[0]
    gt16 = np.full((nb, H + 2, W + 4), -1.0, np.float16)
    gt16[:, 1:H + 1, 2:W + 2] = gt.astype(np.float16)
    sel = _build_sel()
    selc = _build_selc()
    pS = np.ascontiguousarray(np.asarray(preds_S, np.float32).reshape(nb, C, P))
    pT = np.ascontiguousarray(np.asarray(preds_T, np.float32).reshape(nb, C, P))
    return [
        {"preds_s": pS[b], "preds_t": pT[b], "gt16": gt16[b],
         "sel": sel, "selc": selc}
        for b in range(nb)
    ]


def postprocess(gt_labels, partials_per_core) -> np.float32:
    """Reduce per-core [26, 512] partials to the scalar loss."""
    gt = np.asarray(gt_labels)[:, 0]
    nb = gt.shape[0]
    loss = 0.0
    for b in range(nb):
        part = partials_per_core[b].astype(np.float64).sum(axis=1)  # [26]
        g0 = int(gt[b, 0, 0])
        for k in range(1, C):
            n = part[k - 1]
            kls = part[K + k - 1]
            z = 1.0 if g0 == k else 0.0
            if n > z:
                loss += kls / (C * max(n, 1.0))
    return np.float32(loss)


def _run(inputs, trace=False, trace_kwargs=None):
    from concourse.bass_utils import run_bass_kernel_spmd

    nc = _get_nc()
    in_maps = make_in_maps(inputs["preds_S"], inputs["preds_T"],
                           inputs["gt_labels"])
    res = run_bass_kernel_spmd(nc, in_maps, list(range(len(in_maps))),
                               trace=trace, **(trace_kwargs or {}))
    parts = [res.results[b]["partials"] for b in range(len(in_maps))]
    loss = postprocess(inputs["gt_labels"], parts)
    return loss, res


def kernel(preds_S, preds_T, gt_labels):
    assert preds_S.shape == (B, C, H, W), preds_S.shape
    loss, _ = _run({"preds_S": preds_S, "preds_T": preds_T,
                    "gt_labels": gt_labels})
    return loss
